# revision 32
# baseline (speedup 1.0000x reference)
"""Multi-head attention (B=4, S=2048, D=1024, H=16) on 8 TRN2 NeuronCores.

Sharding: batch x head-group (4 batches x 2 groups of 8 heads).  Each core:
  x_b [2048,1024], wq/wk/wv column-slice [1024,512], wo row-slice [512,1024]
  -> partial y [2048,1024]; host sums the two head-group partials per batch
  and adds the folded biases (bo + bv @ wo).

v2 restructure vs the original baseline (429us):
  - host-side bf16 packing of x and weights: half the DMA bytes, no
    on-device CAST, and x is transposed by the DMA xbar
    (dma_start_transpose) instead of 128 PE transposes + DVE copies.
  - weights pre-arranged on host into SBUF layout; per-pair staging DMAs.
  - pair-0 attention starts as soon as its first QK projection block and
    V tiles exist (~15us in), with every remaining projection interleaved
    into the attention stream as 'feed' work; the exp stream then never
    has a large serial prologue (was 104us).
  - phase 2 iterates qb-outer over pairs 1-3 and inlines the U->UT
    transposes + y projection + output DMA per qb, collapsing the 46us
    epilogue to the last qb only.

Per-core dataflow (all SBUF-resident, flash-style attention):
  B. QT = wq.T @ xT + bq ; KT likewise ; V = x @ wv (+ ones column)
  C. per head-pair, per 512-q block, per 128-k tile:
       L^T = KTh_tile.T @ QTh   (K=64 row-tiled pair, auto tile_position)
       E^T = exp(L^T / 8)       (ACT, scale fused)
       U[q,0:65] += E^T_tile.T @ [V_h | 1]  (PSUM accum over k tiles)
     then U[:,0:64] / U[:,64] -> attention out per head
  D. U -> UT via PE transposes ; y = UT.T @ wo ; DMA out
"""

import numpy as np

from concourse import bass, tile, mybir
from concourse.vector_clock import ScopedClock

F32 = mybir.dt.float32
BF16 = mybir.dt.bfloat16
AF = mybir.ActivationFunctionType

N_CORES = 8
S = 2048          # sequence length per core (one batch)
D = 1024          # d_model
DK = 512          # head-group width (8 heads x 64)
NST = S // 128    # 16 seq tiles
NKC = D // 128    # 8 d_model tiles
NMT = DK // 128   # 4 head-pair tiles


def _install_drain_patch():
    """walrus in this image rejects >1 sync-wait per instruction (the limit
    varies by instruction struct; 1 is always safe).  Spread excess waits
    over preceding same-engine nops: same program point, identical
    semantics, a few ns of sequencer issue overhead."""
    import bass_rust

    MAXW = 1
    _orig_add = tile.TileContext._add_instruction

    def _add_split(self, inst):
        si = inst.sync_info
        waits = list(si.on_wait) if si is not None and si.on_wait else []
        if len(waits) > MAXW and inst.engine != mybir.EngineType.Unassigned:
            rest, keep = waits[:-MAXW], waits[-MAXW:]
            while rest:
                nop = mybir.InstNoOp(
                    name=self.nc.get_next_instruction_name(), ins=[], outs=[]
                )
                nop.engine = inst.engine
                nop.sync_info = bass_rust.SyncInfo(
                    on_wait=rest[:MAXW], on_update=[]
                )
                rest = rest[MAXW:]
                _orig_add(self, nop)
            si.on_wait = keep
        _orig_add(self, inst)

    tile.TileContext._add_instruction = _add_split

    def _patched(self, tick_clock, wait_clock):
        probe = self.nc.sync.nop(nofuse=True)
        wait_clock.add_sem_waits(
            probe.ins, ScopedClock({None: tick_clock.global_clock})
        )
        waits = list(probe.ins.sync_info.on_wait or []) if probe.ins.sync_info else []
        if len(waits) > 1:
            probe.ins.sync_info.on_wait = waits[:1]
            rest = waits[1:]
            while rest:
                n = self.nc.sync.nop(nofuse=True)
                n.ins.sync_info = bass_rust.SyncInfo(on_wait=rest[:1], on_update=[])
                rest = rest[1:]
        self.nc.sync.drain()
        self.nc.all_engine_barrier()
        assert self.sems is not None
        popped = self.nc._tile_sem_poison_stack.pop()
        assert popped is self._sem_poison
        self.nc.clear_and_free_semaphores(list(self.sems.allocated().values()))
        self.nc.all_engine_barrier()

    tile.TileContext._drain_and_barrier = _patched


_install_drain_patch()


def build_nc():
    nc = bass.Bass("TRN2", target_bir_lowering=False, debug=False, num_devices=1)
    # xb is pre-transposed on host: [D, S] so xT tiles are plain row DMAs
    xb = nc.dram_tensor("xb", [D, S], BF16, kind="ExternalInput").ap()
    # host pre-arranged: [128, pair, kc, 128] for wq/wk/wv, [128, pair, 1024] wo
    wq = nc.dram_tensor("wq", [128, NMT, NKC, 128], BF16, kind="ExternalInput").ap()
    wk = nc.dram_tensor("wk", [128, NMT, NKC, 128], BF16, kind="ExternalInput").ap()
    wv = nc.dram_tensor("wv", [128, NMT, NKC, 128], BF16, kind="ExternalInput").ap()
    wo = nc.dram_tensor("wo", [128, NMT, D], BF16, kind="ExternalInput").ap()
    bq = nc.dram_tensor("bq", [128, NMT], F32, kind="ExternalInput").ap()
    bk = nc.dram_tensor("bk", [128, NMT], F32, kind="ExternalInput").ap()
    y = nc.dram_tensor("y", [S, D], F32, kind="ExternalOutput").ap()

    with tile.TileContext(nc, pool_alloc_mode="queue") as tc:
        _emit(nc, tc, xb, wq, wk, wv, bq, bk, wo, y)
    return nc


def _emit(nc, tc, xb, wq, wk, wv, bq, bk, wo, y):
    from contextlib import ExitStack

    ctx = ExitStack()
    with ctx:
        consts = ctx.enter_context(tc.tile_pool(name="consts", bufs=1))
        bq_sb = consts.tile([128, NMT], F32, tag="bq")
        bk_sb = consts.tile([128, NMT], F32, tag="bk")

        # ---- persistent tensors ----
        xtp = ctx.enter_context(tc.tile_pool(name="xtp", bufs=1))
        xT = [xtp.tile([128, S], BF16, tag=f"xT{c}", name=f"xT{c}") for c in range(NKC)]
        wst = ctx.enter_context(tc.tile_pool(name="wst", bufs=1))
        wq_sb = wst.tile([128, NMT, NKC, 128], BF16, tag="wq", name="wq")
        wk_sb = wst.tile([128, NMT, NKC, 128], BF16, tag="wk", name="wk")
        wv_sb = wst.tile([128, NMT, NKC, 128], BF16, tag="wv", name="wv")
        wo_sb = wst.tile([128, NMT, D], BF16, tag="wo", name="wo")
        qkv_pool = ctx.enter_context(tc.tile_pool(name="qkv", bufs=1))
        QT = [qkv_pool.tile([128, S], BF16, tag=f"QT{m}", name=f"QT{m}") for m in range(NMT)]
        KT = [qkv_pool.tile([128, S], BF16, tag=f"KT{m}", name=f"KT{m}") for m in range(NMT)]
        # V augmented per head with 64 ones-columns: one full-array matmul
        # [V_h | 1].T @ E_h then yields the numerator U^T on partitions
        # 0:64 and the softmax denominator replicated on partitions 64:128.
        VT = qkv_pool.tile([128, NST, 8, 128], BF16, tag="VT")
        nc.vector.memset(VT[:, :, :, 64:128], 1.0)
        utp = ctx.enter_context(tc.tile_pool(name="utp", bufs=1))
        UT = [utp.tile([128, S], BF16, tag=f"UT{k}", name=f"UT{k}") for k in range(NMT)]

        # ---- phase A: all input DMAs, priority-ordered ----
        for sb in range(4):
            for c in range(NKC):
                nc.sync.dma_start(
                    xT[c][:, sb * 512:(sb + 1) * 512],
                    xb[c * 128:(c + 1) * 128, sb * 512:(sb + 1) * 512],
                )
            if sb == 0:
                nc.sync.dma_start(wq_sb[:, 0], wq[:, 0])
                nc.sync.dma_start(wk_sb[:, 0], wk[:, 0])
                for p in range(NMT):
                    nc.sync.dma_start(wv_sb[:, p], wv[:, p])
                nc.sync.dma_start(bq_sb[:], bq)
                nc.sync.dma_start(bk_sb[:], bk)
            elif sb == 1:
                nc.sync.dma_start(wq_sb[:, 1], wq[:, 1])
                nc.sync.dma_start(wk_sb[:, 1], wk[:, 1])
            elif sb == 2:
                nc.sync.dma_start(wq_sb[:, 2], wq[:, 2])
                nc.sync.dma_start(wk_sb[:, 2], wk[:, 2])
                nc.sync.dma_start(wq_sb[:, 3], wq[:, 3])
                nc.sync.dma_start(wk_sb[:, 3], wk[:, 3])
            else:
                nc.sync.dma_start(wo_sb[:], wo)

        # ---- generators: one instruction per yield ----
        def qk_proj(p, nb, pool):
            """QT/KT projection for pair p, 512-wide seq block nb."""
            for (w_sb, b_sb, dst) in ((wq_sb, bq_sb, QT), (wk_sb, bk_sb, KT)):
                pq = pool.tile([128, 512], F32, tag="pp", name="pq")
                for kc in range(NKC):
                    nc.tensor.matmul(
                        pq[:], w_sb[:, p, kc, :],
                        xT[kc][:, nb * 512:(nb + 1) * 512],
                        start=(kc == 0), stop=(kc == NKC - 1),
                    )
                    yield
                nc.vector.tensor_scalar_add(
                    dst[p][:, nb * 512:(nb + 1) * 512], pq[:], b_sb[:, p:p + 1]
                )
                yield

        def v_proj(st, pool):
            """V projection (all 4 pairs at once) for one 128-seq tile."""
            pv = pool.tile([128, 512], F32, tag="pp", name="pv")
            for kc in range(NKC):
                nc.tensor.matmul(
                    pv[:], xT[kc][:, st * 128:(st + 1) * 128],
                    wv_sb[:, :, kc, :],
                    start=(kc == 0), stop=(kc == NKC - 1),
                )
                yield
            nc.vector.tensor_copy(
                VT[:, st, :, 0:64], pv[:].rearrange("p (h d) -> p h d", h=8)
            )
            yield

        def attention(p, qb, lps, ups, epool, rpool, feed, per_tile):
            """Flash attention for (pair, qb).  AV is one full-array matmul
            per head per k-tile: lhsT = [V_h | ones] [128k, 128], rhs =
            E_h [128k, 512q] -> psum rows 0:64 = U^T numerator (already in
            UT layout), rows 64:128 = sum_k E (denominator, replicated) --
            so normalize is an approx-reciprocal + one elementwise mul."""
            uo = [ups.tile([128, 512], F32, tag="ups", name=f"uo{h}")
                  for h in range(2)]
            for kt in range(NST):
                L = lps.tile([128, 2, 512], F32, tag="L")
                for half in range(2):
                    hsl = slice(half * 64, (half + 1) * 64)
                    nc.tensor.matmul(
                        L[:, half],
                        KT[p][hsl, kt * 128:(kt + 1) * 128],
                        QT[p][hsl, qb * 512:(qb + 1) * 512],
                        start=True, stop=True,
                    )
                E = epool.tile([128, 2, 512], BF16, tag="E")
                nc.scalar.activation(E[:], L[:], AF.Exp, scale=0.125)
                for half in range(2):
                    nc.tensor.matmul(
                        uo[half][:],
                        VT[:, kt, 2 * p + half],
                        E[:, half],
                        start=(kt == 0), stop=(kt == NST - 1),
                        skip_group_check=True,
                    )
                for _ in range(per_tile):
                    if next(feed, None) is None:
                        break
            # Evacuate PSUM fast (frees the uo ring for the next qb), then
            # the slow iterative reciprocal runs from SBUF off the critical
            # path.
            stg = rpool.tile([128, 2, 512], F32, tag="stg")
            for half in range(2):
                nc.vector.tensor_copy(stg[:, half], uo[half][:])
            rec = rpool.tile([128, 2, 512], F32, tag="rec")
            nc.vector.reciprocal(rec[0:64, :, :], stg[64:128, :, :])
            for half in range(2):
                nc.vector.scalar_tensor_tensor(
                    UT[p][half * 64:(half + 1) * 64, qb * 512:(qb + 1) * 512],
                    stg[0:64, half], 1.0, rec[0:64, half],
                    op0=mybir.AluOpType.mult, op1=mybir.AluOpType.mult,
                )

        def tail(qb, ypp, ysp):
            """y projection + output DMA for one qb (all UT ready);
            one instruction per yield."""
            for qt in range(qb * 4, qb * 4 + 4):
                for n in range(2):
                    yp = ypp.tile([128, 512], F32, tag="yp")
                    for kc2 in range(NMT):
                        nc.tensor.matmul(
                            yp[:],
                            UT[kc2][:, qt * 128:(qt + 1) * 128],
                            wo_sb[:, kc2, n * 512:(n + 1) * 512],
                            start=(kc2 == 0), stop=(kc2 == NMT - 1),
                            skip_group_check=True,
                        )
                        yield
                    ys = ysp.tile([128, 512], F32, tag="ys")
                    nc.vector.tensor_copy(ys[:], yp[:])
                    nc.sync.dma_start(
                        y[qt * 128:(qt + 1) * 128, n * 512:(n + 1) * 512], ys[:]
                    )
                    yield

        from itertools import chain

        empty = iter(())
        with tc.tile_pool(name="lps", bufs=2, space="PSUM") as lps, \
             tc.tile_pool(name="ups", bufs=2, space="PSUM") as ups, \
             tc.tile_pool(name="epool", bufs=6) as epool, \
             tc.tile_pool(name="rpool", bufs=4) as rpool:
            # ---- pairs 0-2, p-outer; each pair's attention window feeds
            # the next pair's projections at ~the PE slack rate ----
            with tc.tile_pool(name="ppool", bufs=2, space="PSUM") as ppool:
                # upfront: pair-0 QK over seq block 0, first V tile
                for _ in chain(qk_proj(0, 0, ppool), v_proj(0, ppool)):
                    pass
                # qb0 feed: deadline-checked at per_tile=12 (KT0 nb by
                # kt=4nb, VT st by kt=st)
                feed0 = chain(v_proj(1, ppool), v_proj(2, ppool),
                              v_proj(3, ppool), qk_proj(0, 1, ppool),
                              *[v_proj(st, ppool) for st in range(4, 9)],
                              qk_proj(0, 2, ppool),
                              *[v_proj(st, ppool) for st in range(9, 12)],
                              qk_proj(0, 3, ppool),
                              *[v_proj(st, ppool) for st in range(12, NST)])
                attention(0, 0, lps, ups, epool, rpool, feed0, per_tile=12)
                for _ in feed0:
                    pass
                # pair p's qb feeds pair p+1 projections (72 items / 48
                # tiles -> 1.5/tile avg, matching the PE slack) and does
                # its own U->UT transposes as it goes.
                for p in range(3):
                    feedn = chain(*[qk_proj(p + 1, nb, ppool) for nb in range(4)])
                    for qb in range(4):
                        if p == 0 and qb == 0:
                            continue
                        attention(p, qb, lps, ups, epool, rpool, feedn,
                                  per_tile=(2 if qb % 2 == 0 else 1))
                    for _ in feedn:
                        pass

            # ---- pair 3 qb-outer; previous qb's y proj feeds into the
            # next qb's attention stream ----
            with tc.tile_pool(name="ypp", bufs=2, space="PSUM") as ypp, \
                 tc.tile_pool(name="ysp", bufs=3) as ysp:
                tfeed = empty
                for qb in range(4):
                    attention(3, qb, lps, ups, epool, rpool, tfeed, per_tile=3)
                    for _ in tfeed:
                        pass
                    tfeed = tail(qb, ypp, ysp)
                for _ in tfeed:
                    pass


_NC_CACHE = None


def _get_nc():
    global _NC_CACHE
    if _NC_CACHE is None:
        _NC_CACHE = build_nc()
    return _NC_CACHE


def make_in_maps(x, wq, bq, wk, bk, wv, bv, wo, bo):
    """Shard + host-side pack the full inputs into per-core input maps."""
    import ml_dtypes

    bf16 = ml_dtypes.bfloat16
    x = np.asarray(x, np.float32)
    wq, bq = np.asarray(wq, np.float32), np.asarray(bq, np.float32)
    wk, bk = np.asarray(wk, np.float32), np.asarray(bk, np.float32)
    wv = np.asarray(wv, np.float32)
    wo = np.asarray(wo, np.float32)

    def pack_w(w):  # [1024, 512] -> [128, pair, kc, 128]
        return np.ascontiguousarray(
            w.reshape(NKC, 128, NMT, 128).transpose(1, 2, 0, 3).astype(bf16)
        )

    def pack_wo(w):  # [512, 1024] -> [128, pair, 1024]
        return np.ascontiguousarray(
            w.reshape(NMT, 128, D).transpose(1, 0, 2).astype(bf16)
        )

    def pack_b(b):  # [512] -> [128, pair]
        return np.ascontiguousarray(b.reshape(NMT, 128).T.astype(np.float32))

    in_maps = []
    for core in range(N_CORES):
        b, hg = core // 2, core % 2
        sl = slice(hg * DK, (hg + 1) * DK)
        in_maps.append({
            "xb": np.ascontiguousarray(x[b].T.astype(bf16)),
            "wq": pack_w(wq[:, sl]),
            "wk": pack_w(wk[:, sl]),
            "wv": pack_w(wv[:, sl]),
            "bq": pack_b(bq[sl]),
            "bk": pack_b(bk[sl]),
            "wo": pack_wo(wo[sl, :]),
        })
    return in_maps


def kernel(x, wq, bq, wk, bk, wv, bv, wo, bo):
    from concourse.bass_utils import run_bass_kernel_spmd

    in_maps = make_in_maps(x, wq, bq, wk, bk, wv, bv, wo, bo)
    nc = _get_nc()
    # The first execution after process start can race the input upload on
    # core 0 (observed: garbage/NaN in the earliest-consumed tiles, first
    # run only).  Run once to warm up, then take the second run's results.
    run_bass_kernel_spmd(nc, in_maps, core_ids=list(range(N_CORES)))
    res = run_bass_kernel_spmd(nc, in_maps, core_ids=list(range(N_CORES)))

    wo = np.asarray(wo, np.float32)
    extra = (np.asarray(bo, np.float32) + np.asarray(bv, np.float32) @ wo).astype(
        np.float32
    )
    out = np.empty((4, S, D), np.float32)
    for b in range(4):
        out[b] = res.results[2 * b]["y"] + res.results[2 * b + 1]["y"] + extra
    return out
